# revision 12
# baseline (speedup 1.0000x reference)
"""AQT int8-symmetric quantized dot_general on 8 Trainium2 NeuronCores.

Computes the equivalent of (AQT default int8 config):
    q_lhs, ls = quantize(lhs, axis=K)   # per-row abs-max/127 scales
    q_rhs, rs = quantize(rhs, axis=K)   # per-col abs-max/127 scales
    out = (q_lhs @ q_rhs) * ls * rs     # int32 accumulate, f32 dequant

Sharding: data-parallel over the flattened batch*seq rows of lhs (4096 rows
per core); rhs replicated. No collectives.

Per-core kernel strategy:
  - lhs tiles load in natural [m,K] layout; per-row absmax (VectorE), scale,
    round-to-nearest-even via the +1.5*2^23 trick (ScalarE+VectorE), cast to
    bf16 (integers up to 127 are exact in bf16).
  - quantized tiles are transposed to [K,m] via the DMA x-bar (bf16) to feed
    the TensorEngine's stationary operand.
  - rhs is PE-transposed (f32) so its per-column scales become per-partition;
    quantized the same way, with the dequant scale folded into the bf16
    moving operand.
  - main GEMM: bf16 matmuls accumulating f32 in PSUM (exact for the integer
    lhs side), epilogue applies the lhs scale during the PSUM->SBUF drain.
"""

import numpy as np

import concourse.bass as bass
import concourse.tile as tile
from concourse import bacc, mybir
from concourse.bass_utils import run_bass_kernel_spmd
from concourse.masks import make_identity

N_CORES = 8
K = 1024
N = 1024
M_FULL = 4 * 8192
M_SHARD = M_FULL // N_CORES  # 4096

P = 128                      # partitions
KT = K // P                  # 8 k-chunks
NT = N // P                  # 8 n-chunks (for rhs transpose)
NF = 512                     # moving free dim / PSUM bank
NCH = N // NF                # 2 n-chunks for the main matmul

C_RNE = 12582912.0           # 1.5 * 2**23: (x + C) - C == round-half-even(x)
INV_QB = 1.0 / 127.0
FP32 = mybir.dt.float32
BF16 = mybir.dt.bfloat16
FX = mybir.AxisListType.X


def _body(tc: tile.TileContext, out: bass.AP, lhs: bass.AP, rhs: bass.AP,
          m_shard: int):
    nc = tc.nc
    mt = m_shard // P
    grp = 4 if mt % 4 == 0 else (2 if mt % 2 == 0 else 1)  # m-tiles per DMA
    ng = mt // grp
    with (
        tc.tile_pool(name="const", bufs=1) as constp,
        tc.tile_pool(name="rhsq", bufs=1) as rhsq,
        tc.tile_pool(name="scales", bufs=8) as scp,
        tc.tile_pool(name="mpsum", bufs=6, space="PSUM") as mpsum,
    ):
        ident = constp.tile([P, P], FP32)
        make_identity(nc, ident)

        # QRS[nj][p, kt, nf] = q_rhs[kt*P + p, nj*NF + nf] * s_r[...]
        # split per n-half so main matmuls can start on a half-built rhs
        QRS = [rhsq.tile([P, KT, NF], BF16, tag=f"qrs{nj}", name=f"qrs{nj}")
               for nj in range(NCH)]

        # ---------------- rhs prep (once, scoped pools) ----------------
        with (
            tc.tile_pool(name="rhsp", bufs=1) as rhsp,
            tc.tile_pool(name="rquant", bufs=2) as rquant,
            tc.tile_pool(name="rpsum", bufs=2, space="PSUM") as rpsum,
        ):
            # natural layout: R[p, kt, n] = rhs[kt*P + p, n]
            R = rhsp.tile([P, KT, N], FP32)
            nc.sync.dma_start(R[:], rhs.rearrange("(kt p) n -> p kt n", p=P))

            # PE-transpose to RT[p, nt, k] = rhs[k, nt*P + p], then quantize
            # per n-tile t (rows of RT = original rhs columns) and x-bar the
            # quantized tile back to [k, n] layout.
            RT = rhsp.tile([P, NT, K], FP32)
            tpern = NF // P  # n-tiles per QRS half
            for t in range(NT):
                for j in range(KT):
                    tps = rpsum.tile([P, P], FP32, tag="rtp")
                    nc.tensor.transpose(tps[:], R[:, j, t * P:(t + 1) * P],
                                        ident[:])
                    if j % 2 == 0:
                        nc.scalar.copy(RT[:, t, j * P:(j + 1) * P], tps[:])
                    else:
                        nc.vector.tensor_copy(RT[:, t, j * P:(j + 1) * P],
                                              tps[:])

                rt = RT[:, t, :]
                am_r = scp.tile([P, 1], FP32, tag="am_r")
                nc.vector.tensor_reduce(am_r[:], rt, FX, mybir.AluOpType.max,
                                        apply_absolute_value=True)
                s_r = scp.tile([P, 1], FP32, tag="s_r")
                nc.vector.tensor_scalar(s_r[:], am_r[:], 1e-30, INV_QB,
                                        op0=mybir.AluOpType.max,
                                        op1=mybir.AluOpType.mult)
                inv_r = scp.tile([P, 1], FP32, tag="inv_r")
                nc.vector.reciprocal(inv_r[:], s_r[:])
                pr = rquant.tile([P, K], FP32, tag="pr")
                nc.scalar.activation(pr[:], rt,
                                     mybir.ActivationFunctionType.Copy,
                                     bias=C_RNE, scale=inv_r[:])
                qrs_t = rquant.tile([P, K], BF16, tag="qrs_t")
                nc.vector.tensor_scalar(qrs_t[:], pr[:], -C_RNE, s_r[:],
                                        op0=mybir.AluOpType.add,
                                        op1=mybir.AluOpType.mult)
                # chunked x-bar transpose: out[k, j, n'] = qrs_t[n', j*P + k]
                nc.sync.dma_start_transpose(
                    QRS[t // tpern][:, :, (t % tpern) * P:(t % tpern + 1) * P],
                    qrs_t[:])

        # ---------------- lhs pipeline ----------------
        with (
            tc.tile_pool(name="lload", bufs=3) as lload,
            tc.tile_pool(name="lpass", bufs=3) as lpass,
            tc.tile_pool(name="lq", bufs=4) as lq,
            tc.tile_pool(name="lqt", bufs=4) as lqt,
            tc.tile_pool(name="lout", bufs=3) as lout,
        ):
            for g in range(ng):
                lb = lload.tile([P, grp, K], FP32, tag="lb")
                nc.scalar.dma_start(
                    lb[:],
                    lhs[g * grp * P:(g + 1) * grp * P, :]
                    .rearrange("(t p) k -> p t k", p=P))
                ob = lout.tile([P, grp, N], FP32, tag="ob")

                for ti in range(grp):
                    li = lb[:, ti, :]
                    am = scp.tile([P, 1], FP32, tag="am")
                    nc.vector.tensor_reduce(am[:], li, FX,
                                            mybir.AluOpType.max,
                                            apply_absolute_value=True)
                    s = scp.tile([P, 1], FP32, tag="s")
                    nc.vector.tensor_scalar(s[:], am[:], 1e-30, INV_QB,
                                            op0=mybir.AluOpType.max,
                                            op1=mybir.AluOpType.mult)
                    inv = scp.tile([P, 1], FP32, tag="inv")
                    nc.vector.reciprocal(inv[:], s[:])

                    pi = lpass.tile([P, K], FP32, tag="pi")
                    nc.scalar.activation(pi[:], li,
                                         mybir.ActivationFunctionType.Copy,
                                         bias=C_RNE, scale=inv[:])
                    qi = lq.tile([P, K], BF16, tag="qi")
                    nc.vector.tensor_scalar(qi[:], pi[:], -C_RNE, None,
                                            op0=mybir.AluOpType.add)

                    # one chunked x-bar transpose: qt[k, j, m] = qi[m, j*P+k]
                    qt = lqt.tile([P, KT, P], BF16, tag="qt")
                    nc.sync.dma_start_transpose(qt[:], qi[:])

                    for nj in range(NCH):
                        ps = mpsum.tile([P, NF], FP32, tag="ps")
                        for j in range(KT):
                            nc.tensor.matmul(ps[:], lhsT=qt[:, j, :],
                                             rhs=QRS[nj][:, j, :],
                                             start=(j == 0), stop=(j == KT - 1))
                        # dequant epilogue on the PSUM drain (per-row scale)
                        nc.scalar.activation(ob[:, ti, nj * NF:(nj + 1) * NF],
                                             ps[:],
                                             mybir.ActivationFunctionType.Copy,
                                             bias=0.0, scale=s[:])
                nc.scalar.dma_start(
                    out[g * grp * P:(g + 1) * grp * P, :]
                    .rearrange("(t p) n -> p t n", p=P), ob[:])


_CACHE = {}


def _build(m_shard: int, repeats: int = 1, timing: bool = False) -> bacc.Bacc:
    key = (m_shard, repeats, timing)
    if key in _CACHE:
        return _CACHE[key]
    nc = bacc.Bacc("TRN2", target_bir_lowering=False, debug=False)
    lhs = nc.dram_tensor("lhs", [m_shard, K], FP32, kind="ExternalInput").ap()
    rhs = nc.dram_tensor("rhs", [K, N], FP32, kind="ExternalInput").ap()
    out = nc.dram_tensor("out", [m_shard, N], FP32, kind="ExternalOutput").ap()
    rhs_out = None
    if timing:
        # pass-through copy so timing loops can keep rhs device-resident
        rhs_out = nc.dram_tensor("rhs_out", [K, N], FP32,
                                 kind="ExternalOutput").ap()
    with tile.TileContext(nc) as tc:
        if rhs_out is not None:
            nc.scalar.dma_start(rhs_out[:], rhs[:])
        for _ in range(repeats):
            _body(tc, out, lhs, rhs, m_shard)
    nc.compile()
    _CACHE[key] = nc
    return nc


def kernel(lhs: np.ndarray, rhs: np.ndarray) -> np.ndarray:
    b, sq, k = lhs.shape
    lhs_flat = np.ascontiguousarray(lhs, dtype=np.float32).reshape(b * sq, k)
    rhs = np.ascontiguousarray(rhs, dtype=np.float32)
    m_shard = (b * sq) // N_CORES

    nc = _build(m_shard)
    in_maps = [
        {"lhs": lhs_flat[c * m_shard:(c + 1) * m_shard], "rhs": rhs}
        for c in range(N_CORES)
    ]
    res = run_bass_kernel_spmd(nc, in_maps, core_ids=list(range(N_CORES)))
    outs = [res.results[c]["out"] for c in range(N_CORES)]
    return np.concatenate(outs, axis=0).reshape(b, sq, rhs.shape[1])


# revision 13
# speedup vs baseline: 1.0438x; 1.0438x over previous
"""AQT int8-symmetric quantized dot_general on 8 Trainium2 NeuronCores.

Computes the equivalent of (AQT default int8 config):
    q_lhs, ls = quantize(lhs, axis=K)   # per-row abs-max/127 scales
    q_rhs, rs = quantize(rhs, axis=K)   # per-col abs-max/127 scales
    out = (q_lhs @ q_rhs) * ls * rs     # int32 accumulate, f32 dequant

Sharding: data-parallel over the flattened batch*seq rows of lhs (4096 rows
per core); rhs replicated. No collectives.

Per-core kernel strategy:
  - lhs tiles load in natural [m,K] layout; per-row absmax (VectorE), scale,
    round-to-nearest-even via the +1.5*2^23 trick (ScalarE+VectorE), cast to
    bf16 (integers up to 127 are exact in bf16).
  - quantized tiles are transposed to [K,m] via the DMA x-bar (bf16) to feed
    the TensorEngine's stationary operand.
  - rhs is PE-transposed (f32) so its per-column scales become per-partition;
    quantized the same way, with the dequant scale folded into the bf16
    moving operand.
  - main GEMM: bf16 matmuls accumulating f32 in PSUM (exact for the integer
    lhs side), epilogue applies the lhs scale during the PSUM->SBUF drain.
"""

import numpy as np

import concourse.bass as bass
import concourse.tile as tile
from concourse import bacc, mybir
from concourse.bass_utils import run_bass_kernel_spmd
from concourse.masks import make_identity

N_CORES = 8
K = 1024
N = 1024
M_FULL = 4 * 8192
M_SHARD = M_FULL // N_CORES  # 4096

P = 128                      # partitions
KT = K // P                  # 8 k-chunks
NT = N // P                  # 8 n-chunks (for rhs transpose)
NF = 512                     # moving free dim / PSUM bank
NCH = N // NF                # 2 n-chunks for the main matmul

C_RNE = 12582912.0           # 1.5 * 2**23: (x + C) - C == round-half-even(x)
INV_QB = 1.0 / 127.0
FP32 = mybir.dt.float32
BF16 = mybir.dt.bfloat16
FX = mybir.AxisListType.X


def _body(tc: tile.TileContext, out: bass.AP, lhs: bass.AP, rhs: bass.AP,
          m_shard: int):
    nc = tc.nc
    mt = m_shard // P
    grp = 4 if mt % 4 == 0 else (2 if mt % 2 == 0 else 1)  # m-tiles per DMA
    ng = mt // grp
    with (
        tc.tile_pool(name="const", bufs=1) as constp,
        tc.tile_pool(name="rhsq", bufs=1) as rhsq,
        tc.tile_pool(name="scales", bufs=8) as scp,
        tc.tile_pool(name="mpsum", bufs=6, space="PSUM") as mpsum,
    ):
        ident = constp.tile([P, P], FP32)
        make_identity(nc, ident)

        # QRS[nj][p, kt, nf] = q_rhs[kt*P + p, nj*NF + nf] * s_r[...]
        # split per n-half so main matmuls can start on a half-built rhs
        QRS = [rhsq.tile([P, KT, NF], BF16, tag=f"qrs{nj}", name=f"qrs{nj}")
               for nj in range(NCH)]

        # ---------------- rhs prep (once, scoped pools) ----------------
        with (
            tc.tile_pool(name="rhsp", bufs=1) as rhsp,
            tc.tile_pool(name="rquant", bufs=2) as rquant,
            tc.tile_pool(name="rpsum", bufs=2, space="PSUM") as rpsum,
        ):
            # natural layout: R[p, kt, n] = rhs[kt*P + p, n]
            R = rhsp.tile([P, KT, N], FP32)
            nc.sync.dma_start(R[:], rhs.rearrange("(kt p) n -> p kt n", p=P))

            # PE-transpose to RT[p, nt, k] = rhs[k, nt*P + p], then quantize
            # per n-tile t (rows of RT = original rhs columns) and x-bar the
            # quantized tile back to [k, n] layout.
            RT = rhsp.tile([P, NT, K], FP32)
            tpern = NF // P  # n-tiles per QRS half
            for t in range(NT):
                for j in range(KT):
                    tps = rpsum.tile([P, P], FP32, tag="rtp")
                    nc.tensor.transpose(tps[:], R[:, j, t * P:(t + 1) * P],
                                        ident[:])
                    if j % 2 == 0:
                        nc.scalar.copy(RT[:, t, j * P:(j + 1) * P], tps[:])
                    else:
                        nc.vector.tensor_copy(RT[:, t, j * P:(j + 1) * P],
                                              tps[:])

                rt = RT[:, t, :]
                am_r = scp.tile([P, 1], FP32, tag="am_r")
                nc.vector.tensor_reduce(am_r[:], rt, FX, mybir.AluOpType.max,
                                        apply_absolute_value=True)
                s_r = scp.tile([P, 1], FP32, tag="s_r")
                nc.vector.tensor_scalar(s_r[:], am_r[:], 1e-30, INV_QB,
                                        op0=mybir.AluOpType.max,
                                        op1=mybir.AluOpType.mult)
                inv_r = scp.tile([P, 1], FP32, tag="inv_r")
                nc.vector.reciprocal(inv_r[:], s_r[:])
                pr = rquant.tile([P, K], FP32, tag="pr")
                nc.scalar.activation(pr[:], rt,
                                     mybir.ActivationFunctionType.Copy,
                                     bias=C_RNE, scale=inv_r[:])
                qrs_t = rquant.tile([P, K], BF16, tag="qrs_t")
                nc.vector.tensor_scalar(qrs_t[:], pr[:], -C_RNE, s_r[:],
                                        op0=mybir.AluOpType.add,
                                        op1=mybir.AluOpType.mult)
                # chunked x-bar transpose: out[k, j, n'] = qrs_t[n', j*P + k]
                nc.sync.dma_start_transpose(
                    QRS[t // tpern][:, :, (t % tpern) * P:(t % tpern + 1) * P],
                    qrs_t[:])

        # ---------------- lhs pipeline ----------------
        with (
            tc.tile_pool(name="lload", bufs=3) as lload,
            tc.tile_pool(name="lpass", bufs=3) as lpass,
            tc.tile_pool(name="lq", bufs=4) as lq,
            tc.tile_pool(name="lqt", bufs=4) as lqt,
            tc.tile_pool(name="lout", bufs=3) as lout,
        ):
            for g in range(ng):
                lb = lload.tile([P, grp, K], FP32, tag="lb")
                nc.scalar.dma_start(
                    lb[:],
                    lhs[g * grp * P:(g + 1) * grp * P, :]
                    .rearrange("(t p) k -> p t k", p=P))
                ob = lout.tile([P, grp, N], FP32, tag="ob")

                # group-batched absmax / scales: one op per group
                am = scp.tile([P, grp], FP32, tag="am")
                nc.vector.tensor_reduce(am[:], lb[:], FX, mybir.AluOpType.max,
                                        apply_absolute_value=True)
                s = scp.tile([P, grp], FP32, tag="s")
                nc.vector.tensor_scalar(s[:], am[:], 1e-30, INV_QB,
                                        op0=mybir.AluOpType.max,
                                        op1=mybir.AluOpType.mult)
                inv = scp.tile([P, grp], FP32, tag="inv")
                nc.vector.reciprocal(inv[:], s[:])

                # pass1 per tile (activation scale is per-partition only)
                pb = lpass.tile([P, grp, K], FP32, tag="pb")
                for ti in range(grp):
                    nc.scalar.activation(pb[:, ti, :], lb[:, ti, :],
                                         mybir.ActivationFunctionType.Copy,
                                         bias=C_RNE, scale=inv[:, ti:ti + 1])
                # pass2 + x-bar transpose batched over the whole group
                qb = lq.tile([P, grp, K], BF16, tag="qb")
                nc.vector.tensor_scalar(qb[:], pb[:], -C_RNE, None,
                                        op0=mybir.AluOpType.add)
                qt = lqt.tile([P, grp, KT, P], BF16, tag="qt")
                # out[k, (ti j), m] = qb[m, (ti j)*P + k]
                nc.sync.dma_start_transpose(
                    qt[:].rearrange("p t j m -> p (t j) m"),
                    qb[:].rearrange("p t k -> p (t k)"))

                for ti in range(grp):
                    for nj in range(NCH):
                        ps = mpsum.tile([P, NF], FP32, tag="ps")
                        for j in range(KT):
                            nc.tensor.matmul(ps[:], lhsT=qt[:, ti, j, :],
                                             rhs=QRS[nj][:, j, :],
                                             start=(j == 0), stop=(j == KT - 1))
                        # dequant epilogue on the PSUM drain (per-row scale)
                        nc.scalar.activation(ob[:, ti, nj * NF:(nj + 1) * NF],
                                             ps[:],
                                             mybir.ActivationFunctionType.Copy,
                                             bias=0.0, scale=s[:, ti:ti + 1])
                nc.scalar.dma_start(
                    out[g * grp * P:(g + 1) * grp * P, :]
                    .rearrange("(t p) n -> p t n", p=P), ob[:])


_CACHE = {}


def _build(m_shard: int, repeats: int = 1, timing: bool = False) -> bacc.Bacc:
    key = (m_shard, repeats, timing)
    if key in _CACHE:
        return _CACHE[key]
    nc = bacc.Bacc("TRN2", target_bir_lowering=False, debug=False)
    lhs = nc.dram_tensor("lhs", [m_shard, K], FP32, kind="ExternalInput").ap()
    rhs = nc.dram_tensor("rhs", [K, N], FP32, kind="ExternalInput").ap()
    out = nc.dram_tensor("out", [m_shard, N], FP32, kind="ExternalOutput").ap()
    rhs_out = None
    if timing:
        # pass-through copy so timing loops can keep rhs device-resident
        rhs_out = nc.dram_tensor("rhs_out", [K, N], FP32,
                                 kind="ExternalOutput").ap()
    with tile.TileContext(nc) as tc:
        if rhs_out is not None:
            nc.scalar.dma_start(rhs_out[:], rhs[:])
        for _ in range(repeats):
            _body(tc, out, lhs, rhs, m_shard)
    nc.compile()
    _CACHE[key] = nc
    return nc


def kernel(lhs: np.ndarray, rhs: np.ndarray) -> np.ndarray:
    b, sq, k = lhs.shape
    lhs_flat = np.ascontiguousarray(lhs, dtype=np.float32).reshape(b * sq, k)
    rhs = np.ascontiguousarray(rhs, dtype=np.float32)
    m_shard = (b * sq) // N_CORES

    nc = _build(m_shard)
    in_maps = [
        {"lhs": lhs_flat[c * m_shard:(c + 1) * m_shard], "rhs": rhs}
        for c in range(N_CORES)
    ]
    res = run_bass_kernel_spmd(nc, in_maps, core_ids=list(range(N_CORES)))
    outs = [res.results[c]["out"] for c in range(N_CORES)]
    return np.concatenate(outs, axis=0).reshape(b, sq, rhs.shape[1])


# revision 38
# speedup vs baseline: 1.0646x; 1.0200x over previous
"""AQT int8-symmetric quantized dot_general on 8 Trainium2 NeuronCores.

Computes the equivalent of (AQT default int8 config):
    q_lhs, ls = quantize(lhs, axis=K)   # per-row abs-max/127 scales
    q_rhs, rs = quantize(rhs, axis=K)   # per-col abs-max/127 scales
    out = (q_lhs @ q_rhs) * ls * rs     # int32 accumulate, f32 dequant

Sharding: data-parallel over the flattened batch*seq rows of lhs (4096 rows
per core); rhs replicated. No collectives.

Per-core kernel strategy:
  - lhs tiles load in natural [m,K] layout; per-row absmax (VectorE), scale,
    round-to-nearest-even via the +1.5*2^23 trick (ScalarE+VectorE), cast to
    bf16 (integers up to 127 are exact in bf16).
  - quantized tiles are transposed to [K,m] via the DMA x-bar (bf16) to feed
    the TensorEngine's stationary operand.
  - rhs is PE-transposed (f32) so its per-column scales become per-partition;
    quantized the same way, with the dequant scale folded into the bf16
    moving operand.
  - main GEMM: bf16 matmuls accumulating f32 in PSUM (exact for the integer
    lhs side), epilogue applies the lhs scale during the PSUM->SBUF drain.
"""

import numpy as np

import concourse.bass as bass
import concourse.tile as tile
from concourse import bacc, mybir
from concourse.bass_utils import run_bass_kernel_spmd
from concourse.masks import make_identity

N_CORES = 8
K = 1024
N = 1024
M_FULL = 4 * 8192
M_SHARD = M_FULL // N_CORES  # 4096

P = 128                      # partitions
KT = K // P                  # 8 k-chunks
NT = N // P                  # 8 n-chunks (for rhs transpose)
NF = 512                     # moving free dim / PSUM bank
NCH = N // NF                # 2 n-chunks for the main matmul

C_RNE = 12582912.0           # 1.5 * 2**23: (x + C) - C == round-half-even(x)
INV_QB = 1.0 / 127.0
FP32 = mybir.dt.float32
BF16 = mybir.dt.bfloat16
FX = mybir.AxisListType.X


def _body(tc: tile.TileContext, out: bass.AP, lhs: bass.AP, rhs: bass.AP,
          m_shard: int):
    nc = tc.nc
    mt = m_shard // P
    grp = 4 if mt % 4 == 0 else (2 if mt % 2 == 0 else 1)  # m-tiles per DMA
    ng = mt // grp
    with (
        tc.tile_pool(name="const", bufs=1) as constp,
        tc.tile_pool(name="rhsq", bufs=1) as rhsq,
        tc.tile_pool(name="scales", bufs=8) as scp,
        tc.tile_pool(name="mpsum", bufs=4, space="PSUM") as mpsum,
        tc.tile_pool(name="lload", bufs=2) as lload,
        tc.tile_pool(name="lpass", bufs=2) as lpass,
        tc.tile_pool(name="lq", bufs=6) as lq,
        tc.tile_pool(name="lqt", bufs=6) as lqt,
        tc.tile_pool(name="lout", bufs=3) as lout,
    ):
        lbs = {}
        def load_group(g):
            lb = lload.tile([P, grp, K], FP32, tag="lb", name=f"lb{g}")
            nc.gpsimd.dma_start(
                lb[:],
                lhs[g * grp * P:(g + 1) * grp * P, :]
                .rearrange("(t p) k -> p t k", p=P))
            lbs[g] = lb

        # R first (it heads the rhs critical chain), then the first lhs
        # loads so the lhs quant pipeline fills while rhs is being prepared
        R = rhsq.tile([P, KT, N], FP32, name="R")
        nc.sync.dma_start(R[:], rhs.rearrange("(kt p) n -> p kt n", p=P))
        load_group(0)
        if ng > 1:
            load_group(1)

        ident = constp.tile([P, P], FP32)
        make_identity(nc, ident)

        # QRS[nj][p, kt, nf] = q_rhs[kt*P + p, nj*NF + nf] * s_r[...]
        # split per n-half so main matmuls can start on a half-built rhs
        QRS = [rhsq.tile([P, KT, NF], BF16, tag=f"qrs{nj}", name=f"qrs{nj}")
               for nj in range(NCH)]

        # ---------------- rhs prep + lhs pipeline ----------------
        with (
            tc.tile_pool(name="rtrow", bufs=4) as rtrow,
            tc.tile_pool(name="rquant", bufs=3) as rquant,
            tc.tile_pool(name="rpsum", bufs=4, space="PSUM") as rpsum,
        ):
            # PE-transpose one n-tile t at a time: rt[p, k] = rhs[k, t*P + p],
            # quantize its rows (original rhs columns), x-bar back to [k, n].
            tpern = NF // P  # n-tiles per QRS half

            def rhs_chain(t):
                rt = rtrow.tile([P, K], FP32, tag="rt", name=f"rt{t}")
                for j in range(KT):
                    tps = rpsum.tile([P, P], FP32, tag="rtp", name=f"rtp{t}_{j}")
                    nc.tensor.transpose(tps[:], R[:, j, t * P:(t + 1) * P],
                                        ident[:])
                    if j % 2 == 0:
                        nc.scalar.copy(rt[:, j * P:(j + 1) * P], tps[:])
                    else:
                        nc.vector.tensor_copy(rt[:, j * P:(j + 1) * P],
                                              tps[:])

                am_r = scp.tile([P, 1], FP32, tag="am_r", name=f"am_r{t}")
                nc.vector.tensor_reduce(am_r[:], rt[:], FX,
                                        mybir.AluOpType.max,
                                        apply_absolute_value=True)
                s_r = scp.tile([P, 1], FP32, tag="s_r", name=f"s_r{t}")
                nc.vector.tensor_scalar(s_r[:], am_r[:], 1e-30, INV_QB,
                                        op0=mybir.AluOpType.max,
                                        op1=mybir.AluOpType.mult)
                inv_r = scp.tile([P, 1], FP32, tag="inv_r", name=f"inv_r{t}")
                nc.vector.reciprocal(inv_r[:], s_r[:])
                pr = rquant.tile([P, K], FP32, tag="pr", name=f"pr{t}")
                nc.scalar.activation(pr[:], rt[:],
                                     mybir.ActivationFunctionType.Copy,
                                     bias=C_RNE, scale=inv_r[:])
                qrs_t = rquant.tile([P, K], BF16, tag="qrs_t", name=f"qt_r{t}")
                nc.vector.tensor_scalar(qrs_t[:], pr[:], -C_RNE, s_r[:],
                                        op0=mybir.AluOpType.add,
                                        op1=mybir.AluOpType.mult)
                # chunked x-bar transpose: out[k, j, n'] = qrs_t[n', j*P + k]
                nc.sync.dma_start_transpose(
                    QRS[t // tpern][:, :, (t % tpern) * P:(t % tpern + 1) * P],
                    qrs_t[:])

            # rhs fully prepared ahead of the lhs compute in program order
            for t in range(NT):
                rhs_chain(t)

            for g in range(ng):
                if g + 2 < ng:
                    load_group(g + 2)
                lb = lbs.pop(g)
                ob = lout.tile([P, grp, N], FP32, tag="ob")

                # group-batched absmax / scales: one op per group
                am = scp.tile([P, grp], FP32, tag="am")
                nc.vector.tensor_reduce(am[:], lb[:], FX, mybir.AluOpType.max,
                                        apply_absolute_value=True)
                s = scp.tile([P, grp], FP32, tag="s")
                nc.vector.tensor_scalar(s[:], am[:], 1e-30, INV_QB,
                                        op0=mybir.AluOpType.max,
                                        op1=mybir.AluOpType.mult)
                inv = scp.tile([P, grp], FP32, tag="inv")
                nc.vector.reciprocal(inv[:], s[:])

                for ti in range(grp):
                    # pass1 per tile (activation scale is per-partition only)
                    pi = lpass.tile([P, K], FP32, tag="pi")
                    nc.scalar.activation(pi[:], lb[:, ti, :],
                                         mybir.ActivationFunctionType.Copy,
                                         bias=C_RNE, scale=inv[:, ti:ti + 1])
                    qi = lq.tile([P, K], BF16, tag="qi")
                    nc.vector.tensor_scalar(qi[:], pi[:], -C_RNE, None,
                                            op0=mybir.AluOpType.add)
                    # chunked x-bar transpose: qt[k, j, m] = qi[m, j*P + k]
                    qt = lqt.tile([P, KT, P], BF16, tag="qt")
                    nc.sync.dma_start_transpose(qt[:], qi[:])

                    for nj in range(NCH):
                        ps = mpsum.tile([P, NF], FP32, tag="ps")
                        for j in range(KT):
                            nc.tensor.matmul(ps[:], lhsT=qt[:, j, :],
                                             rhs=QRS[nj][:, j, :],
                                             start=(j == 0), stop=(j == KT - 1))
                        # dequant epilogue on the PSUM drain (per-row scale)
                        nc.scalar.activation(ob[:, ti, nj * NF:(nj + 1) * NF],
                                             ps[:],
                                             mybir.ActivationFunctionType.Copy,
                                             bias=0.0, scale=s[:, ti:ti + 1])
                nc.scalar.dma_start(
                    out[g * grp * P:(g + 1) * grp * P, :]
                    .rearrange("(t p) n -> p t n", p=P), ob[:])


_CACHE = {}


def _build(m_shard: int, repeats: int = 1, timing: bool = False) -> bacc.Bacc:
    key = (m_shard, repeats, timing)
    if key in _CACHE:
        return _CACHE[key]
    nc = bacc.Bacc("TRN2", target_bir_lowering=False, debug=False)
    lhs = nc.dram_tensor("lhs", [m_shard, K], FP32, kind="ExternalInput").ap()
    rhs = nc.dram_tensor("rhs", [K, N], FP32, kind="ExternalInput").ap()
    out = nc.dram_tensor("out", [m_shard, N], FP32, kind="ExternalOutput").ap()
    rhs_out = None
    if timing:
        # pass-through copy so timing loops can keep rhs device-resident
        rhs_out = nc.dram_tensor("rhs_out", [K, N], FP32,
                                 kind="ExternalOutput").ap()
    with tile.TileContext(nc) as tc:
        if rhs_out is not None:
            nc.scalar.dma_start(rhs_out[:], rhs[:])
        for _ in range(repeats):
            _body(tc, out, lhs, rhs, m_shard)
    nc.compile()
    _CACHE[key] = nc
    return nc


def kernel(lhs: np.ndarray, rhs: np.ndarray) -> np.ndarray:
    b, sq, k = lhs.shape
    lhs_flat = np.ascontiguousarray(lhs, dtype=np.float32).reshape(b * sq, k)
    rhs = np.ascontiguousarray(rhs, dtype=np.float32)
    m_shard = (b * sq) // N_CORES

    nc = _build(m_shard)
    in_maps = [
        {"lhs": lhs_flat[c * m_shard:(c + 1) * m_shard], "rhs": rhs}
        for c in range(N_CORES)
    ]
    res = run_bass_kernel_spmd(nc, in_maps, core_ids=list(range(N_CORES)))
    outs = [res.results[c]["out"] for c in range(N_CORES)]
    return np.concatenate(outs, axis=0).reshape(b, sq, rhs.shape[1])
